# revision 53
# baseline (speedup 1.0000x reference)
"""HadamardTest kernel for Trainium2, 8-core data-parallel SPMD.

out[n, c] = (z_re @ refT)^2 + (z_im @ refT)^2, ref = L2-normalized zero-padded canon.

Sharding: z_re/z_im split along samples into 8 shards of 2048 rows; the tiny
normalized 1024x10 refT table is replicated. Each core computes its
[10, 2048] slice of the (transposed) output.

Device algorithm per core (bf16 inputs, fp32 accumulation):
  - the host downcasts z to bf16 (halves HBM traffic; rel err ~2.5e-3, well
    inside the 2e-2 gate),
  - z^T chunk tiles [128, 2048] arrive straight from DRAM via XBAR
    DMA-transpose (sync queue only - scalar-queue XBAR DMAs race), so there
    are no PE transposes, no PSUM staging and no copies,
  - bf16 matmuls with the refT chunk stationary accumulate <z|ref> into 8
    PSUM banks (4 blocks x re/im) across the dk sweep,
  - epilogue: Act squares (PSUM->SBUF fused), DVE adds, DMA out [10, 2048].

kernel() caches the compiled SPMD executable after the first call; every call
still transfers the full inputs, executes on all 8 cores, and fetches the
full output.
"""
import numpy as np
import ml_dtypes

import concourse.mybir as mybir
import concourse.tile as tile
from concourse import bacc
from concourse._compat import axon_active

F32 = mybir.dt.float32
BF16 = mybir.dt.bfloat16
AF = mybir.ActivationFunctionType

N = 16384          # total samples
DIM = 1024         # state dimension (2**10)
C = 10             # classes
IMG = 784          # 28*28 pixels before zero-pad
N_CORES = 8
NS = N // N_CORES  # 2048 samples per core
NT = 512           # matmul moving free dim (PSUM bank limit)
NB = NS // NT      # 4 blocks per core
KCH = DIM // 128   # 8 contraction chunks
P = 128

_CACHE = {}


def build_kernel(repeat=None):
    key = ("nc", repeat)
    if key in _CACHE:
        return _CACHE[key]
    nc = bacc.Bacc(None, target_bir_lowering=False, debug=False,
                   num_devices=N_CORES)
    zre_d = nc.dram_tensor("z_re", [KCH, NS, P], BF16, kind="ExternalInput").ap()
    zim_d = nc.dram_tensor("z_im", [KCH, NS, P], BF16, kind="ExternalInput").ap()
    refT_d = nc.dram_tensor("refT", [DIM, C], BF16, kind="ExternalInput").ap()
    outT_d = nc.dram_tensor("outT", [C, NS], F32, kind="ExternalOutput").ap()

    with tile.TileContext(nc) as tc:
        with (
            tc.tile_pool(name="const", bufs=1) as cpool,
            tc.tile_pool(name="ztr", bufs=4) as ztpool,
            tc.tile_pool(name="outsb", bufs=2) as opool,
            tc.tile_pool(name="opsum", bufs=1, space="PSUM") as opsum,
        ):
            rt = cpool.tile([P, KCH, C], BF16)
            nc.sync.dma_start(
                out=rt[:], in_=refT_d.rearrange("(k p) c -> p k c", p=P))

            for _rep in range(repeat or 1):
                # 8 accumulators (4 blocks x re/im) live across the dk sweep:
                # exactly the 8 PSUM banks
                ps = [[opsum.tile([C, NT], F32, tag=f"po{b}_{pi}",
                                  name=f"ps{b}{pi}")
                       for pi in range(2)] for b in range(NB)]
                os_ = [opool.tile([C, NT], F32, tag=f"o{b}", name=f"ob{b}")
                       for b in range(NB)]
                # pi-outer: the re accumulators finish half-way through, so
                # their squares overlap the im sweep instead of piling into
                # a serial tail when everything completes at once
                for pi, zd in enumerate((zre_d, zim_d)):
                    for dk in range(KCH):
                        # one XBAR transpose-load of the whole shard's chunk:
                        # [2048, 128] bf16 -> [128, 2048]
                        zt = ztpool.tile([P, NS], BF16, tag="zt")
                        nc.sync.dma_start_transpose(out=zt[:], in_=zd[dk])
                        for b in range(NB):
                            nc.tensor.matmul(
                                ps[b][pi][:], rt[:, dk],
                                zt[:, b * NT:(b + 1) * NT],
                                start=(dk == 0), stop=(dk == KCH - 1))
                    if pi == 0:
                        for b in range(NB):
                            nc.scalar.activation(out=os_[b][:],
                                                 in_=ps[b][0][:],
                                                 func=AF.Square)
                for b in range(NB):
                    t2 = opool.tile([C, NT], F32, tag="t2", name="t2")
                    nc.scalar.activation(out=t2[:], in_=ps[b][1][:],
                                         func=AF.Square)
                    nc.vector.tensor_add(out=os_[b][:], in0=os_[b][:],
                                         in1=t2[:])
                    nc.sync.dma_start(
                        out=outT_d[:, b * NT:(b + 1) * NT], in_=os_[b][:])

    nc.finalize()
    _CACHE[key] = nc
    return nc


def _swizzle(z):
    # per-core [NS, DIM] -> [KCH, NS, 128]: each chunk contiguous in DRAM,
    # so the XBAR transpose reads dense 256B rows (~2-3us faster than the
    # 2KB-strided column slices)
    return np.ascontiguousarray(z.reshape(-1, KCH, P).transpose(1, 0, 2))


def _swizzle_full(z):
    # full [N, DIM] -> concat-ready [N_CORES*KCH, NS, 128]
    return np.ascontiguousarray(
        z.reshape(N_CORES, NS, KCH, P).transpose(0, 2, 1, 3)
        .reshape(N_CORES * KCH, NS, P))


def _prep_host(z_re, z_im, canon):
    ref = np.asarray(canon, dtype=np.float32).reshape(C, IMG)
    ref = np.pad(ref, ((0, 0), (0, DIM - IMG)))
    ref = ref / np.linalg.norm(ref, axis=1, keepdims=True)
    refT = np.ascontiguousarray(ref.T).astype(ml_dtypes.bfloat16)
    z_re = np.asarray(z_re, dtype=np.float32).astype(ml_dtypes.bfloat16)
    z_im = np.asarray(z_im, dtype=np.float32).astype(ml_dtypes.bfloat16)
    return z_re, z_im, refT


def prepare_in_maps(z_re, z_im, canon):
    z_re, z_im, refT = _prep_host(z_re, z_im, canon)
    return [
        {
            "z_re": _swizzle(z_re[c * NS:(c + 1) * NS]),
            "z_im": _swizzle(z_im[c * NS:(c + 1) * NS]),
            "refT": refT,
        }
        for c in range(N_CORES)
    ]


class _Runner:
    """Cached compiled SPMD callable over device-sharded inputs (axon/PJRT).

    Mirrors concourse.bass2jax.run_bass_via_pjrt's lowering, but compiles
    once (fast-dispatch, no donation) and is reused across kernel() calls.
    Input/output avals are derived from the BIR allocations.
    """

    def __init__(self, nc):
        import jax
        from jax.sharding import Mesh, PartitionSpec, NamedSharding
        from jax.experimental.shard_map import shard_map
        from concourse.bass2jax import (
            _bass_exec_p, fast_dispatch_compile, install_neuronx_cc_hook,
            partition_id_tensor,
        )

        install_neuronx_cc_hook()
        self.jax = jax
        partition_name = (nc.partition_id_tensor.name
                          if nc.partition_id_tensor else None)
        in_specs_np, out_names, out_avals, zero_outs = [], [], [], []
        in_names = []
        for alloc in nc.m.functions[0].allocations:
            if not isinstance(alloc, mybir.MemoryLocationSet):
                continue
            name = alloc.memorylocations[0].name
            shape = tuple(alloc.tensor_shape or ())
            dtype = mybir.dt.np(alloc.dtype) if alloc.dtype is not None else None
            if alloc.kind == "ExternalInput":
                if name != partition_name:
                    in_names.append(name)
                    in_specs_np.append((shape, dtype))
            elif alloc.kind == "ExternalOutput":
                out_names.append(name)
                out_avals.append(jax.core.ShapedArray(shape, dtype))
                zero_outs.append(np.zeros(shape, dtype))
        self.in_names, self.out_names = in_names, out_names
        self.out_avals = out_avals
        all_in = list(in_names) + list(out_names)
        if partition_name is not None:
            all_in.append(partition_name)

        def _body(*args):
            operands = list(args)
            if partition_name is not None:
                operands.append(partition_id_tensor())
            return tuple(_bass_exec_p.bind(
                *operands,
                out_avals=tuple(out_avals),
                in_names=tuple(all_in),
                out_names=tuple(out_names),
                lowering_input_output_aliases=(),
                sim_require_finite=True,
                sim_require_nnan=True,
                nc=nc,
            ))

        devices = jax.devices()[:N_CORES]
        mesh = Mesh(np.asarray(devices), ("core",))
        spec = PartitionSpec("core")
        self.sharding = NamedSharding(mesh, spec)
        n_args = len(in_names) + len(out_names)
        self.zero_concat = [
            jax.device_put(
                np.zeros((N_CORES * z.shape[0], *z.shape[1:]), z.dtype),
                self.sharding)
            for z in zero_outs
        ]
        example = tuple(
            jax.ShapeDtypeStruct((N_CORES * s[0], *s[1:]), dt,
                                 sharding=self.sharding)
            for s, dt in in_specs_np
        ) + tuple(
            jax.ShapeDtypeStruct(z.shape, z.dtype, sharding=self.sharding)
            for z in self.zero_concat
        )
        assert len(example) == n_args

        def _compile():
            return (
                jax.jit(
                    shard_map(_body, mesh=mesh, in_specs=(spec,) * n_args,
                              out_specs=(spec,) * len(out_names),
                              check_rep=False),
                    keep_unused=True,
                )
                .lower(*example)
                .compile()
            )

        self.fn = fast_dispatch_compile(_compile)

    def __call__(self, z_re, z_im, refT):
        jax = self.jax
        put = lambda x: jax.device_put(x, self.sharding)
        args = (
            put(_swizzle_full(z_re)),
            put(_swizzle_full(z_im)),
            put(np.ascontiguousarray(
                np.broadcast_to(refT, (N_CORES,) + refT.shape)
                .reshape(N_CORES * DIM, C))),
        ) + tuple(self.zero_concat)
        outs = self.fn(*args)
        outT = np.asarray(outs[0]).reshape(N_CORES, C, NS)   # [8, 10, 2048]
        return np.ascontiguousarray(
            outT.transpose(0, 2, 1).reshape(N, C))


_RUNNER = None


def kernel(z_re, z_im, canon):
    z_re, z_im, refT = _prep_host(z_re, z_im, canon)
    if axon_active():
        global _RUNNER
        if _RUNNER is None:
            _RUNNER = _Runner(build_kernel())
        return _RUNNER(z_re, z_im, refT)
    # native (non-axon) fallback: one-shot run via the stock SPMD driver
    from concourse.bass_utils import run_bass_kernel_spmd
    nc = build_kernel()
    in_maps = prepare_in_maps(z_re, z_im, canon)
    res = run_bass_kernel_spmd(nc, in_maps, list(range(N_CORES)), trace=False)
    out = np.empty((N, C), dtype=np.float32)
    for c in range(N_CORES):
        out[c * NS:(c + 1) * NS] = res.results[c]["outT"].T
    return out
